# revision 25
# baseline (speedup 1.0000x reference)
"""PointNet feature-propagation block on 8 Trainium2 NeuronCores.

Data-parallel over the batch dim: 16 batches -> 2 per core.
Per batch on-device pipeline:
  1. G = feat @ W1^T (+b1) per sparse point (PE, fp16 inputs); rows
     [512 x fp16 | xs coords as 3 x fp32 | pad] (1280 B) stored to HBM.
  2. Scores S = -d2 via augmented K=15 split-fp16 matmul (PE): each fp32
     factor is decomposed into fp16 hi+lo (coords) / hi+mid+lo (norms), so
     products keep ~fp32 accuracy at 1 cycle/row instead of 4:
       2 xd.xs = 2xdh*xsh + 2xdh*xsl + 2xdl*xsh   (lo*lo dropped, <2^-24)
  3. Top-8 candidates per dense point: DVE max + max_index; keep 4.
  4. dma_gather of the 4 candidate G-rows (with coords) per dense point.
  5. Exact d2 recomputed from gathered coords (scores alone lose ~3e-7
     absolute, far too coarse for the closest pairs); the worst of the 4
     candidates is excluded by a zero weight; w_m = (1/(sqrt(d2_m)+eps))
     normalized.
  6. t_m = w_m * g_m: four tensor_scalar (4x DVE mode on fp16); the sum
     happens for free in PSUM: the four PE transposes per 128-column block
     accumulate into one f32 bank.  Relu rides the ACT PSUM->SBUF copy.
  7. out = h @ W2^T (+b2) (PE), stored as (dense, ch) fp16.
"""
import numpy as np

import concourse.bass as bass
import concourse.tile as tile
import concourse.mybir as mybir
from concourse import bacc
from concourse.bass_utils import run_bass_kernel_spmd

B, N1, N2 = 16, 1024, 4096
C_IN, C_OUT = 512, 512
NCAND = 4                    # candidates gathered per dense point
EPS = 1e-10
N_CORES = 8
BPC = B // N_CORES           # batches per core
NCH = N2 // 128              # dense chunks per batch (32)
NR = N1 // 128               # sparse chunks (8)
NQI = C_IN // 128            # input-channel chunks (4)
NQO = C_OUT // 128           # output-channel chunks (4)
JQ = 8                       # dense chunks per gather group
NGRP = NCH // JQ             # 4 gather groups
KS = 15                      # score-matmul contraction rows (split fp16)
GROW = 640                   # G row length in fp16 elems (512 feat + 6 coord + pad)

F32 = mybir.dt.float32
F16 = mybir.dt.float16
U16 = mybir.dt.uint16
I16 = mybir.dt.int16
Alu = mybir.AluOpType
Act = mybir.ActivationFunctionType
AxX = mybir.AxisListType.X


def _emit_front(nc, pools, aps, b, include_b1):
    (sb, gpool, ixpool, opool, wpool, fpool, wsump, htp,
     psum_s, psum_g, psum_t, psum_o) = pools
    (xdT, xsT, featT, xd_pc, xs_pc, w1T_sb, w2T_sb, b1row_sb, b2row_sb,
     ones16_sb, identT_sb, out_ap, g_dram) = aps

    # ---- Stage 2 first: scores + top-8 (the DVE critical path starts on
    # the first score matmul; stage-1's PE work fills PE idle behind it) ----
    xdT_sb = sb.tile([KS, N2], F16, tag="xdT")
    nc.sync.dma_start(xdT_sb[:], xdT[b])
    xsT_sb = sb.tile([KS, N1], F16, tag="xsT")
    nc.scalar.dma_start(xsT_sb[:], xsT[b])
    xd_pc_sb = fpool.tile([128, NCH, 3], F32, tag="xd_pc")
    nc.scalar.dma_start(xd_pc_sb[:], xd_pc[b])

    m8 = fpool.tile([128, NCH, 8], F32, tag="m8")
    i8 = fpool.tile([128, NCH, 8], U16, tag="i8")
    for c in range(NCH):
        ps = psum_s.tile([128, N1], F32)
        lhs = xdT_sb[:, c * 128:(c + 1) * 128]
        nc.tensor.matmul(ps[:, 0:512], lhs, xsT_sb[:, 0:512], start=True, stop=True)
        nc.tensor.matmul(ps[:, 512:1024], lhs, xsT_sb[:, 512:1024], start=True, stop=True)
        nc.vector.max(m8[:, c, :], ps[:])
        nc.vector.max_index(i8[:, c, :], m8[:, c, :], ps[:])

    # ---- Stage 1: G rows = [feat @ W1^T (+b1) | xs coords | pad] -----------
    featT_sb = sb.tile([128, NQI, N1], F16, tag="featT")
    for q in range(NQI):
        nc.scalar.dma_start(featT_sb[:, q, :], featT[b, q * 128:(q + 1) * 128, :])
    xs_pc_sb = sb.tile([128, NR, 3], F32, tag="xs_pc")
    nc.sync.dma_start(xs_pc_sb[:], xs_pc[b])

    g_sb = sb.tile([128, NR, GROW], F16, tag="g_sb")
    for r in range(NR):
        pg = psum_g.tile([128, C_OUT], F32)
        for q in range(NQI):
            nc.tensor.matmul(
                pg[:], featT_sb[:, q, r * 128:(r + 1) * 128], w1T_sb[:, q, :],
                start=(q == 0), stop=(q == NQI - 1 and not include_b1))
        if include_b1:
            nc.tensor.matmul(pg[:], ones16_sb[0:1, :], b1row_sb[:],
                             start=False, stop=True)
        nc.scalar.activation(g_sb[:, r, 0:C_OUT], pg[:], Act.Copy)
    # coords + zero pad
    nc.vector.tensor_copy(g_sb[:, :, C_OUT:C_OUT + 6].bitcast(F32), xs_pc_sb[:])
    nc.vector.memset(g_sb[:, :, C_OUT + 6:GROW], 0.0)
    # store to HBM with row-major (1024, GROW) layout: row r*128+p
    nc.scalar.dma_start(g_dram[b].rearrange("(r p) e -> p r e", p=128), g_sb[:])
    return m8, i8, xd_pc_sb


def _emit_idxw(nc, pools, b, front):
    (sb, gpool, ixpool, opool, wpool, fpool, wsump, htp,
     psum_s, psum_g, psum_t, psum_o) = pools
    m8, i8, xd_pc_sb = front

    # ---- Stage 3: candidate index lists in wrapped int16 layout ------------
    idxw = ixpool.tile([128, NCAND, N2 // 16], I16, tag="idxw")
    for m in range(NCAND):
        # dst[p%16, c*8 + p//16] = i8[p, c, m]
        dstv = idxw[0:16, m, :].rearrange("q (c pp) -> q c pp", pp=8)
        for pp in range(8):
            nc.sync.dma_start(dstv[:, :, pp],
                              i8[16 * pp:16 * (pp + 1), :, m].bitcast(I16))
    for r in range(1, 8):
        nc.sync.dma_start(idxw[16 * r:16 * (r + 1), :, :], idxw[0:16, :, :])
    return idxw


def _emit_group(nc, pools, aps, b, front, idxw, Q, include_b2):
    (sb, gpool, ixpool, opool, wpool, fpool, wsump, htp,
     psum_s, psum_g, psum_t, psum_o) = pools
    (xdT, xsT, featT, xd_pc, xs_pc, w1T_sb, w2T_sb, b1row_sb, b2row_sb,
     ones16_sb, identT_sb, out_ap, g_dram) = aps
    m8, i8, xd_pc_sb = front

    js = slice(Q * JQ, (Q + 1) * JQ)
    gk = gpool.tile([128, NCAND, JQ, GROW], F16, tag="gkall", name="gkall")
    for m in range(NCAND):
        nc.gpsimd.dma_gather(
            out_ap=gk[:, m, :, :], in_ap=g_dram[b],
            idxs_ap=idxw[:, m, Q * (JQ * 128 // 16):(Q + 1) * (JQ * 128 // 16)],
            num_idxs=JQ * 128, num_idxs_reg=JQ * 128, elem_size=GROW)

    # exact d2 for all 4 candidates fused (layout [m, jj]); reductions
    # over m use a strided [jj, m] view of the same memory
    cview = gk[:, :, :, C_OUT:C_OUT + 6].bitcast(F32)   # [128, m, jj, 3]
    diff = wpool.tile([128, NCAND, JQ, 3], F32, tag="cdiff")
    nc.vector.tensor_tensor(diff[:], cview,
                            xd_pc_sb[:, js, :].unsqueeze(1)
                            .broadcast_to([128, NCAND, JQ, 3]),
                            Alu.subtract)
    sq = wpool.tile([128, NCAND, JQ, 3], F32, tag="csq")
    nc.vector.tensor_tensor(sq[:], diff[:], diff[:], Alu.mult)
    d2q = wpool.tile([128, NCAND, JQ], F32, tag="d2q")
    nc.vector.tensor_reduce(d2q[:].unsqueeze(3), sq[:], AxX, Alu.add)
    d2v = d2q[:].rearrange("p m j -> p j m")            # strided view

    # weights: u = 1/(sqrt(d2)+eps); exclude the max-d2 candidate; norm
    dmax = wpool.tile([128, JQ, 1], F32, tag="dmax")
    nc.vector.tensor_reduce(dmax[:], d2v, AxX, Alu.max)
    keep = wpool.tile([128, NCAND, JQ], F32, tag="keep")
    nc.vector.tensor_tensor(keep[:].rearrange("p m j -> p j m"), d2v,
                            dmax[:].broadcast_to([128, JQ, NCAND]), Alu.is_lt)
    dist = wpool.tile([128, NCAND, JQ], F32, tag="cdist")
    nc.scalar.activation(dist[:], d2q[:], Act.Sqrt, bias=EPS * EPS)
    u = wpool.tile([128, NCAND, JQ], F32, tag="cu")
    nc.vector.reciprocal(u[:], dist[:])
    u0 = wpool.tile([128, NCAND, JQ], F32, tag="cu0")
    nc.vector.tensor_tensor(u0[:], u[:], keep[:], Alu.mult)
    usum = wpool.tile([128, JQ, 1], F32, tag="cusum")
    nc.vector.tensor_reduce(usum[:], u0[:].rearrange("p m j -> p j m"),
                            AxX, Alu.add)
    sf = wpool.tile([128, JQ, 1], F32, tag="csf")
    nc.vector.reciprocal(sf[:], usum[:])
    w = wpool.tile([128, NCAND, JQ], F32, tag="cw")
    nc.vector.tensor_tensor(w[:].rearrange("p m j -> p j m"),
                            u0[:].rearrange("p m j -> p j m"),
                            sf[:].broadcast_to([128, JQ, NCAND]), Alu.mult)

    for jj in range(JQ):
        c = Q * JQ + jj
        # h^T = sum_m g_m^T @ diag(w_m): the weighting, the transpose AND
        # the candidate sum all happen on the PE with f32 PSUM accumulation.
        # DVE only builds the four tiny diagonal matrices (ident * w_m,
        # per-partition scalar, 128 elems in 4x mode).
        dw = []
        for m in range(NCAND):
            d = wsump.tile([128, 128], F16, tag="dw")
            nc.vector.tensor_scalar(d[:], identT_sb[:],
                                    scalar1=w[:, m, jj:jj + 1],
                                    op0=Alu.mult, scalar2=1.0, op1=Alu.mult)
            dw.append(d)
        pt = psum_t.tile([128, NQO, 128], F32)
        for q in range(NQO):
            for m in range(NCAND):
                nc.tensor.matmul(pt[:, q, :],
                                 gk[:, m, jj, q * 128:(q + 1) * 128],
                                 dw[m][:],
                                 start=(m == 0), stop=(m == NCAND - 1))
        hT2 = htp.tile([128, NQO, 128], F16, tag="hT2")
        nc.scalar.activation(hT2[:], pt[:], Act.Relu)
        po = psum_o.tile([128, C_OUT], F32)
        for q in range(NQO):
            nc.tensor.matmul(
                po[:], hT2[:, q, :], w2T_sb[:, q, :],
                start=(q == 0), stop=(q == NQO - 1 and not include_b2))
        if include_b2:
            nc.tensor.matmul(po[:], ones16_sb[0:1, :], b2row_sb[:],
                             start=False, stop=True)
        osb = opool.tile([128, C_OUT], F16, tag="osb")
        nc.scalar.activation(osb[:], po[:], Act.Copy)
        nc.sync.dma_start(out_ap[b, c * 128:(c + 1) * 128, :], osb[:])


def _build(include_b1, include_b2, reps=1):
    nc = bacc.Bacc("TRN2", target_bir_lowering=False, debug=False,
                   num_devices=N_CORES)

    # register EPS^2 as a const AP so it can be an ACT Sqrt bias
    # (sqrt(d2 + EPS^2) == sqrt(d2) + EPS at fp32 for all reachable d2)
    _ct = nc.alloc_sbuf_tensor("const-float32-epssq", [128, 1], F32)
    nc.gpsimd.memset(_ct.ap(), EPS * EPS)
    nc.const_aps.aps[(F32, EPS * EPS)] = _ct.ap()
    nc.all_engine_barrier()

    xdT = nc.dram_tensor("xdT", [BPC, KS, N2], F16, kind="ExternalInput").ap()
    xsT = nc.dram_tensor("xsT", [BPC, KS, N1], F16, kind="ExternalInput").ap()
    featT = nc.dram_tensor("featT", [BPC, C_IN, N1], F16, kind="ExternalInput").ap()
    xd_pc = nc.dram_tensor("xd_pc", [BPC, 128, NCH, 3], F32, kind="ExternalInput").ap()
    xs_pc = nc.dram_tensor("xs_pc", [BPC, 128, NR, 3], F32, kind="ExternalInput").ap()
    w1T = nc.dram_tensor("w1T", [C_IN, C_OUT], F16, kind="ExternalInput").ap()
    w2T = nc.dram_tensor("w2T", [C_IN, C_OUT], F16, kind="ExternalInput").ap()
    b1row = nc.dram_tensor("b1row", [1, C_OUT], F16, kind="ExternalInput").ap()
    b2row = nc.dram_tensor("b2row", [1, C_OUT], F16, kind="ExternalInput").ap()
    ident = nc.dram_tensor("ident", [128, 128], F16, kind="ExternalInput").ap()
    out_ap = nc.dram_tensor("out", [BPC, N2, C_OUT], F16, kind="ExternalOutput").ap()
    g_drams = [nc.dram_tensor(f"gscratch{r}", [BPC, N1, GROW], F16).ap()
               for r in range(reps)]

    with tile.TileContext(nc) as tc:
        with (
            tc.tile_pool(name="sb", bufs=2) as sb,
            tc.tile_pool(name="gpool", bufs=2) as gpool,
            tc.tile_pool(name="ixpool", bufs=2) as ixpool,
            tc.tile_pool(name="opool", bufs=4) as opool,
            tc.tile_pool(name="wpool", bufs=2) as wpool,
            tc.tile_pool(name="fpool", bufs=2) as fpool,
            tc.tile_pool(name="wsump", bufs=12) as wsump,
            tc.tile_pool(name="htp", bufs=3) as htp,
            tc.tile_pool(name="const", bufs=1) as constp,
            tc.tile_pool(name="psum_s", bufs=2, space="PSUM") as psum_s,
            tc.tile_pool(name="psum_g", bufs=1, space="PSUM") as psum_g,
            tc.tile_pool(name="psum_t", bufs=1, space="PSUM") as psum_t,
            tc.tile_pool(name="psum_o", bufs=2, space="PSUM") as psum_o,
        ):
            w1T_sb = constp.tile([128, NQI, C_OUT], F16, tag="w1T")
            for q in range(NQI):
                nc.scalar.dma_start(w1T_sb[:, q, :], w1T[q * 128:(q + 1) * 128, :])
            w2T_sb = constp.tile([128, NQI, C_OUT], F16, tag="w2T")
            for q in range(NQI):
                nc.scalar.dma_start(w2T_sb[:, q, :], w2T[q * 128:(q + 1) * 128, :])
            b1row_sb = constp.tile([1, C_OUT], F16, tag="b1row")
            b2row_sb = constp.tile([1, C_OUT], F16, tag="b2row")
            ones16_sb = constp.tile([1, 128], F16, tag="ones16")
            identT_sb = constp.tile([128, 128], F16, tag="identT")
            nc.scalar.dma_start(identT_sb[:], ident[:])
            if include_b1:
                nc.vector.memset(ones16_sb[:], 1.0)
                nc.sync.dma_start(b1row_sb[:], b1row[:])
            if include_b2:
                nc.vector.memset(ones16_sb[:], 1.0)
                nc.sync.dma_start(b2row_sb[:], b2row[:])

            pools = (sb, gpool, ixpool, opool, wpool, fpool, wsump, htp,
                     psum_s, psum_g, psum_t, psum_o)
            for _rep in range(reps):
                aps = (xdT, xsT, featT, xd_pc, xs_pc, w1T_sb, w2T_sb,
                       b1row_sb, b2row_sb, ones16_sb, identT_sb, out_ap,
                       g_drams[_rep])
                # interleave per batch: batch b's groups (PE/ACT/DMA-heavy,
                # barely any DVE) overlap batch b+1's front (DVE-bound
                # top-k), because each engine runs its stream in emission
                # order.
                for b in range(BPC):
                    front = _emit_front(nc, pools, aps, b, include_b1)
                    idxw = _emit_idxw(nc, pools, b, front)
                    for Q in range(NGRP):
                        _emit_group(nc, pools, aps, b, front, idxw, Q,
                                    include_b2)

    nc.compile()
    return nc


_CACHE = {}


def _get_module(include_b1, include_b2, reps=1):
    key = (include_b1, include_b2, reps)
    if key not in _CACHE:
        _CACHE[key] = _build(include_b1, include_b2, reps)
    return _CACHE[key]


def _split16(x):
    hi = x.astype(np.float16)
    lo = (x - hi.astype(np.float32)).astype(np.float16)
    return hi, lo


def _three_split16(x64):
    h1 = x64.astype(np.float16)
    r = x64 - h1.astype(np.float64)
    h2 = r.astype(np.float16)
    h3 = (r - h2.astype(np.float64)).astype(np.float16)
    return h1, h2, h3


def make_in_maps(xyz_dense, xyz_sparse, feat_sparse, W1, b1, W2, b2):
    xd = np.asarray(xyz_dense, np.float32)
    xs = np.asarray(xyz_sparse, np.float32)
    feat = np.asarray(feat_sparse, np.float32)

    # split-fp16 augmented score factors: S = 2 xd.xs - |xd|^2 - |xs|^2 = -d2
    # row pairing (lhs k <-> rhs k), 15 rows:
    #   k=0..2 coords c: (2xdh_c, xsh_c), (2xdh_c, xsl_c), (2xdl_c, xsh_c)
    #   9..11: (-nd_{h,m,l}, 1);  12..14: (1, -ns_{h,m,l})
    xdh, xdl = _split16(xd)
    xsh, xsl = _split16(xs)
    nd2 = np.sum(xd.astype(np.float64) ** 2, -1)
    ns2 = np.sum(xs.astype(np.float64) ** 2, -1)
    ndh, ndm, ndl = _three_split16(nd2)
    nsh, nsm, nsl = _three_split16(ns2)

    xdT = np.empty((B, KS, N2), np.float16)
    xsT = np.empty((B, KS, N1), np.float16)
    for c in range(3):
        xdT[:, 3 * c + 0] = 2.0 * xdh[:, :, c]
        xdT[:, 3 * c + 1] = 2.0 * xdh[:, :, c]
        xdT[:, 3 * c + 2] = 2.0 * xdl[:, :, c]
        xsT[:, 3 * c + 0] = xsh[:, :, c]
        xsT[:, 3 * c + 1] = xsl[:, :, c]
        xsT[:, 3 * c + 2] = xsh[:, :, c]
    xdT[:, 9], xdT[:, 10], xdT[:, 11] = -ndh, -ndm, -ndl
    xdT[:, 12:15] = 1.0
    xsT[:, 9:12] = 1.0
    xsT[:, 12], xsT[:, 13], xsT[:, 14] = -nsh, -nsm, -nsl

    featT = np.ascontiguousarray(feat.transpose(0, 2, 1)).astype(np.float16)
    # partition-major coords: [p, chunk, 3] with point index = chunk*128 + p
    xd_pc = np.ascontiguousarray(xd.reshape(B, NCH, 128, 3).transpose(0, 2, 1, 3))
    xs_pc = np.ascontiguousarray(xs.reshape(B, NR, 128, 3).transpose(0, 2, 1, 3))
    w1T = np.ascontiguousarray(np.asarray(W1, np.float32).T.astype(np.float16))
    w2T = np.ascontiguousarray(np.asarray(W2, np.float32).T.astype(np.float16))
    b1row = np.asarray(b1, np.float32).astype(np.float16).reshape(1, C_OUT)
    b2row = np.asarray(b2, np.float32).astype(np.float16).reshape(1, C_OUT)
    identm = np.eye(128, dtype=np.float16)

    in_maps = []
    for core in range(N_CORES):
        s = slice(core * BPC, (core + 1) * BPC)
        in_maps.append({
            "xdT": np.ascontiguousarray(xdT[s]),
            "xsT": np.ascontiguousarray(xsT[s]),
            "featT": np.ascontiguousarray(featT[s]),
            "xd_pc": np.ascontiguousarray(xd_pc[s]),
            "xs_pc": np.ascontiguousarray(xs_pc[s]),
            "w1T": w1T, "w2T": w2T, "b1row": b1row, "b2row": b2row,
            "ident": identm,
        })
    return in_maps


def kernel(xyz_dense, xyz_sparse, feat_sparse, W1, b1, W2, b2):
    include_b1 = bool(np.any(np.asarray(b1) != 0))
    include_b2 = bool(np.any(np.asarray(b2) != 0))
    nc = _get_module(include_b1, include_b2)
    in_maps = make_in_maps(xyz_dense, xyz_sparse, feat_sparse, W1, b1, W2, b2)
    res = run_bass_kernel_spmd(nc, in_maps, list(range(N_CORES)))
    out = np.concatenate([res.results[i]["out"] for i in range(N_CORES)], axis=0)
    return np.ascontiguousarray(out.astype(np.float32))


# revision 27
# speedup vs baseline: 1.0395x; 1.0395x over previous
"""PointNet feature-propagation block on 8 Trainium2 NeuronCores.

Data-parallel over the batch dim: 16 batches -> 2 per core.
Per batch on-device pipeline:
  1. G = feat @ W1^T (+b1) per sparse point (PE, fp16 inputs); rows
     [512 x fp16 | xs coords as 3 x fp32 | pad] (1280 B) stored to HBM.
  2. Scores S = -d2 via augmented K=15 split-fp16 matmul (PE): each fp32
     factor is decomposed into fp16 hi+lo (coords) / hi+mid+lo (norms), so
     products keep ~fp32 accuracy at 1 cycle/row instead of 4:
       2 xd.xs = 2xdh*xsh + 2xdh*xsl + 2xdl*xsh   (lo*lo dropped, <2^-24)
  3. Top-8 candidates per dense point: DVE max + max_index; keep 4.
  4. dma_gather of the 4 candidate G-rows (with coords) per dense point.
  5. Exact d2 recomputed from gathered coords (scores alone lose ~3e-7
     absolute, far too coarse for the closest pairs); the worst of the 4
     candidates is excluded by a zero weight; w_m = (1/(sqrt(d2_m)+eps))
     normalized.
  6. t_m = w_m * g_m: four tensor_scalar (4x DVE mode on fp16); the sum
     happens for free in PSUM: the four PE transposes per 128-column block
     accumulate into one f32 bank.  Relu rides the ACT PSUM->SBUF copy.
  7. out = h @ W2^T (+b2) (PE), stored as (dense, ch) fp16.
"""
import numpy as np

import concourse.bass as bass
import concourse.tile as tile
import concourse.mybir as mybir
from concourse import bacc
from concourse.bass_utils import run_bass_kernel_spmd

B, N1, N2 = 16, 1024, 4096
C_IN, C_OUT = 512, 512
NCAND = 4                    # candidates gathered per dense point
EPS = 1e-10
N_CORES = 8
BPC = B // N_CORES           # batches per core
NCH = N2 // 128              # dense chunks per batch (32)
NR = N1 // 128               # sparse chunks (8)
NQI = C_IN // 128            # input-channel chunks (4)
NQO = C_OUT // 128           # output-channel chunks (4)
JQ = 8                       # dense chunks per gather group
NGRP = NCH // JQ             # 4 gather groups
KS = 15                      # score-matmul contraction rows (split fp16)
GROW = 640                   # G row length in fp16 elems (512 feat + 6 coord + pad)

F32 = mybir.dt.float32
F16 = mybir.dt.float16
U16 = mybir.dt.uint16
I16 = mybir.dt.int16
Alu = mybir.AluOpType
Act = mybir.ActivationFunctionType
AxX = mybir.AxisListType.X


def _emit_front(nc, pools, aps, b, include_b1):
    (sb, gpool, ixpool, opool, wpool, fpool, wsump, htp,
     psum_s, psum_g, psum_t, psum_o) = pools
    (xdT, xsT, featT, xd_pc, xs_pc, w1T_sb, w2T_sb, b1row_sb, b2row_sb,
     ones16_sb, identT_sb, out_ap, g_dram) = aps

    # ---- Stage 2 first: scores + top-8 (the DVE critical path starts on
    # the first score matmul; stage-1's PE work fills PE idle behind it) ----
    xdT_sb = sb.tile([KS, N2], F16, tag="xdT")
    nc.sync.dma_start(xdT_sb[:], xdT[b])
    xsT_sb = sb.tile([KS, N1], F16, tag="xsT")
    nc.scalar.dma_start(xsT_sb[:], xsT[b])
    xd_pc_sb = fpool.tile([128, NCH, 3], F32, tag="xd_pc")
    nc.scalar.dma_start(xd_pc_sb[:], xd_pc[b])

    m8 = fpool.tile([128, NCH, 8], F32, tag="m8")
    i8 = fpool.tile([128, NCH, 8], U16, tag="i8")
    for c in range(NCH):
        ps = psum_s.tile([128, N1], F32)
        lhs = xdT_sb[:, c * 128:(c + 1) * 128]
        nc.tensor.matmul(ps[:, 0:512], lhs, xsT_sb[:, 0:512], start=True, stop=True)
        nc.tensor.matmul(ps[:, 512:1024], lhs, xsT_sb[:, 512:1024], start=True, stop=True)
        nc.vector.max(m8[:, c, :], ps[:])
        nc.vector.max_index(i8[:, c, :], m8[:, c, :], ps[:])

    # ---- Stage 1: G rows = [feat @ W1^T (+b1) | xs coords | pad] -----------
    featT_sb = sb.tile([128, NQI, N1], F16, tag="featT")
    for q in range(NQI):
        nc.scalar.dma_start(featT_sb[:, q, :], featT[b, q * 128:(q + 1) * 128, :])
    xs_pc_sb = sb.tile([128, NR, 3], F32, tag="xs_pc")
    nc.sync.dma_start(xs_pc_sb[:], xs_pc[b])

    g_sb = sb.tile([128, NR, GROW], F16, tag="g_sb")
    for r in range(NR):
        pg = psum_g.tile([128, C_OUT], F32)
        for q in range(NQI):
            nc.tensor.matmul(
                pg[:], featT_sb[:, q, r * 128:(r + 1) * 128], w1T_sb[:, q, :],
                start=(q == 0), stop=(q == NQI - 1 and not include_b1))
        if include_b1:
            nc.tensor.matmul(pg[:], ones16_sb[0:1, :], b1row_sb[:],
                             start=False, stop=True)
        nc.scalar.activation(g_sb[:, r, 0:C_OUT], pg[:], Act.Copy)
    # coords + zero pad
    nc.vector.tensor_copy(g_sb[:, :, C_OUT:C_OUT + 6].bitcast(F32), xs_pc_sb[:])
    nc.vector.memset(g_sb[:, :, C_OUT + 6:GROW], 0.0)
    # store to HBM with row-major (1024, GROW) layout: row r*128+p
    nc.scalar.dma_start(g_dram[b].rearrange("(r p) e -> p r e", p=128), g_sb[:])
    return m8, i8, xd_pc_sb


def _emit_idxw(nc, pools, b, front):
    (sb, gpool, ixpool, opool, wpool, fpool, wsump, htp,
     psum_s, psum_g, psum_t, psum_o) = pools
    m8, i8, xd_pc_sb = front

    # ---- Stage 3: candidate index lists in wrapped int16 layout ------------
    idxw = ixpool.tile([128, NCAND, N2 // 16], I16, tag="idxw")
    for m in range(NCAND):
        # dst[p%16, c*8 + p//16] = i8[p, c, m]
        dstv = idxw[0:16, m, :].rearrange("q (c pp) -> q c pp", pp=8)
        for pp in range(8):
            nc.sync.dma_start(dstv[:, :, pp],
                              i8[16 * pp:16 * (pp + 1), :, m].bitcast(I16))
    for sz in (16, 32, 64):
        nc.sync.dma_start(idxw[sz:2 * sz, :, :], idxw[0:sz, :, :])
    return idxw


def _emit_group(nc, pools, aps, b, front, idxw, Q, include_b2):
    (sb, gpool, ixpool, opool, wpool, fpool, wsump, htp,
     psum_s, psum_g, psum_t, psum_o) = pools
    (xdT, xsT, featT, xd_pc, xs_pc, w1T_sb, w2T_sb, b1row_sb, b2row_sb,
     ones16_sb, identT_sb, out_ap, g_dram) = aps
    m8, i8, xd_pc_sb = front

    js = slice(Q * JQ, (Q + 1) * JQ)
    gk = gpool.tile([128, NCAND, JQ, GROW], F16, tag="gkall", name="gkall")
    for m in range(NCAND):
        nc.gpsimd.dma_gather(
            out_ap=gk[:, m, :, :], in_ap=g_dram[b],
            idxs_ap=idxw[:, m, Q * (JQ * 128 // 16):(Q + 1) * (JQ * 128 // 16)],
            num_idxs=JQ * 128, num_idxs_reg=JQ * 128, elem_size=GROW)

    # exact d2 for all 4 candidates fused (layout [m, jj]); reductions
    # over m use a strided [jj, m] view of the same memory
    cview = gk[:, :, :, C_OUT:C_OUT + 6].bitcast(F32)   # [128, m, jj, 3]
    diff = wpool.tile([128, NCAND, JQ, 3], F32, tag="cdiff")
    nc.vector.tensor_tensor(diff[:], cview,
                            xd_pc_sb[:, js, :].unsqueeze(1)
                            .broadcast_to([128, NCAND, JQ, 3]),
                            Alu.subtract)
    sq = wpool.tile([128, NCAND, JQ, 3], F32, tag="csq")
    nc.vector.tensor_tensor(sq[:], diff[:], diff[:], Alu.mult)
    d2q = wpool.tile([128, NCAND, JQ], F32, tag="d2q")
    nc.vector.tensor_reduce(d2q[:].unsqueeze(3), sq[:], AxX, Alu.add)
    d2v = d2q[:].rearrange("p m j -> p j m")            # strided view

    # weights: u = 1/(sqrt(d2)+eps); exclude the max-d2 candidate; norm
    dmax = wpool.tile([128, JQ, 1], F32, tag="dmax")
    nc.vector.tensor_reduce(dmax[:], d2v, AxX, Alu.max)
    keep = wpool.tile([128, NCAND, JQ], F32, tag="keep")
    nc.vector.tensor_tensor(keep[:].rearrange("p m j -> p j m"), d2v,
                            dmax[:].broadcast_to([128, JQ, NCAND]), Alu.is_lt)
    dist = wpool.tile([128, NCAND, JQ], F32, tag="cdist")
    nc.scalar.activation(dist[:], d2q[:], Act.Sqrt, bias=EPS * EPS)
    u = wpool.tile([128, NCAND, JQ], F32, tag="cu")
    nc.vector.reciprocal(u[:], dist[:])
    u0 = wpool.tile([128, NCAND, JQ], F32, tag="cu0")
    nc.vector.tensor_tensor(u0[:], u[:], keep[:], Alu.mult)
    usum = wpool.tile([128, JQ, 1], F32, tag="cusum")
    nc.vector.tensor_reduce(usum[:], u0[:].rearrange("p m j -> p j m"),
                            AxX, Alu.add)
    sf = wpool.tile([128, JQ, 1], F32, tag="csf")
    nc.vector.reciprocal(sf[:], usum[:])
    w = wpool.tile([128, NCAND, JQ], F32, tag="cw")
    nc.vector.tensor_tensor(w[:].rearrange("p m j -> p j m"),
                            u0[:].rearrange("p m j -> p j m"),
                            sf[:].broadcast_to([128, JQ, NCAND]), Alu.mult)

    for jj in range(JQ):
        c = Q * JQ + jj
        # h^T = sum_m g_m^T @ diag(w_m): the weighting, the transpose AND
        # the candidate sum all happen on the PE with f32 PSUM accumulation.
        # DVE only builds the four tiny diagonal matrices (ident * w_m,
        # per-partition scalar, 128 elems in 4x mode).
        dw = []
        for m in range(NCAND):
            d = wsump.tile([128, 128], F16, tag="dw")
            nc.vector.tensor_scalar(d[:], identT_sb[:],
                                    scalar1=w[:, m, jj:jj + 1],
                                    op0=Alu.mult, scalar2=1.0, op1=Alu.mult)
            dw.append(d)
        pt = psum_t.tile([128, NQO, 128], F32)
        for q in range(NQO):
            for m in range(NCAND):
                nc.tensor.matmul(pt[:, q, :],
                                 gk[:, m, jj, q * 128:(q + 1) * 128],
                                 dw[m][:],
                                 start=(m == 0), stop=(m == NCAND - 1))
        hT2 = htp.tile([128, NQO, 128], F16, tag="hT2")
        nc.scalar.activation(hT2[:], pt[:], Act.Relu)
        po = psum_o.tile([128, C_OUT], F32)
        for q in range(NQO):
            nc.tensor.matmul(
                po[:], hT2[:, q, :], w2T_sb[:, q, :],
                start=(q == 0), stop=(q == NQO - 1 and not include_b2))
        if include_b2:
            nc.tensor.matmul(po[:], ones16_sb[0:1, :], b2row_sb[:],
                             start=False, stop=True)
        osb = opool.tile([128, C_OUT], F16, tag="osb")
        nc.scalar.activation(osb[:], po[:], Act.Copy)
        nc.sync.dma_start(out_ap[b, c * 128:(c + 1) * 128, :], osb[:])


def _build(include_b1, include_b2, reps=1):
    nc = bacc.Bacc("TRN2", target_bir_lowering=False, debug=False,
                   num_devices=N_CORES)

    # register EPS^2 as a const AP so it can be an ACT Sqrt bias
    # (sqrt(d2 + EPS^2) == sqrt(d2) + EPS at fp32 for all reachable d2)
    _ct = nc.alloc_sbuf_tensor("const-float32-epssq", [128, 1], F32)
    nc.gpsimd.memset(_ct.ap(), EPS * EPS)
    nc.const_aps.aps[(F32, EPS * EPS)] = _ct.ap()
    nc.all_engine_barrier()

    xdT = nc.dram_tensor("xdT", [BPC, KS, N2], F16, kind="ExternalInput").ap()
    xsT = nc.dram_tensor("xsT", [BPC, KS, N1], F16, kind="ExternalInput").ap()
    featT = nc.dram_tensor("featT", [BPC, C_IN, N1], F16, kind="ExternalInput").ap()
    xd_pc = nc.dram_tensor("xd_pc", [BPC, 128, NCH, 3], F32, kind="ExternalInput").ap()
    xs_pc = nc.dram_tensor("xs_pc", [BPC, 128, NR, 3], F32, kind="ExternalInput").ap()
    w1T = nc.dram_tensor("w1T", [C_IN, C_OUT], F16, kind="ExternalInput").ap()
    w2T = nc.dram_tensor("w2T", [C_IN, C_OUT], F16, kind="ExternalInput").ap()
    b1row = nc.dram_tensor("b1row", [1, C_OUT], F16, kind="ExternalInput").ap()
    b2row = nc.dram_tensor("b2row", [1, C_OUT], F16, kind="ExternalInput").ap()
    ident = nc.dram_tensor("ident", [128, 128], F16, kind="ExternalInput").ap()
    out_ap = nc.dram_tensor("out", [BPC, N2, C_OUT], F16, kind="ExternalOutput").ap()
    g_drams = [nc.dram_tensor(f"gscratch{r}", [BPC, N1, GROW], F16).ap()
               for r in range(reps)]

    with tile.TileContext(nc) as tc:
        with (
            tc.tile_pool(name="sb", bufs=2) as sb,
            tc.tile_pool(name="gpool", bufs=2) as gpool,
            tc.tile_pool(name="ixpool", bufs=2) as ixpool,
            tc.tile_pool(name="opool", bufs=4) as opool,
            tc.tile_pool(name="wpool", bufs=2) as wpool,
            tc.tile_pool(name="fpool", bufs=2) as fpool,
            tc.tile_pool(name="wsump", bufs=12) as wsump,
            tc.tile_pool(name="htp", bufs=3) as htp,
            tc.tile_pool(name="const", bufs=1) as constp,
            tc.tile_pool(name="psum_s", bufs=2, space="PSUM") as psum_s,
            tc.tile_pool(name="psum_g", bufs=1, space="PSUM") as psum_g,
            tc.tile_pool(name="psum_t", bufs=2, space="PSUM") as psum_t,
            tc.tile_pool(name="psum_o", bufs=1, space="PSUM") as psum_o,
        ):
            w1T_sb = constp.tile([128, NQI, C_OUT], F16, tag="w1T")
            for q in range(NQI):
                nc.scalar.dma_start(w1T_sb[:, q, :], w1T[q * 128:(q + 1) * 128, :])
            w2T_sb = constp.tile([128, NQI, C_OUT], F16, tag="w2T")
            for q in range(NQI):
                nc.scalar.dma_start(w2T_sb[:, q, :], w2T[q * 128:(q + 1) * 128, :])
            b1row_sb = constp.tile([1, C_OUT], F16, tag="b1row")
            b2row_sb = constp.tile([1, C_OUT], F16, tag="b2row")
            ones16_sb = constp.tile([1, 128], F16, tag="ones16")
            identT_sb = constp.tile([128, 128], F16, tag="identT")
            nc.scalar.dma_start(identT_sb[:], ident[:])
            if include_b1:
                nc.vector.memset(ones16_sb[:], 1.0)
                nc.sync.dma_start(b1row_sb[:], b1row[:])
            if include_b2:
                nc.vector.memset(ones16_sb[:], 1.0)
                nc.sync.dma_start(b2row_sb[:], b2row[:])

            pools = (sb, gpool, ixpool, opool, wpool, fpool, wsump, htp,
                     psum_s, psum_g, psum_t, psum_o)
            for _rep in range(reps):
                aps = (xdT, xsT, featT, xd_pc, xs_pc, w1T_sb, w2T_sb,
                       b1row_sb, b2row_sb, ones16_sb, identT_sb, out_ap,
                       g_drams[_rep])
                # interleave per batch: batch b's groups (PE/ACT/DMA-heavy,
                # barely any DVE) overlap batch b+1's front (DVE-bound
                # top-k), because each engine runs its stream in emission
                # order.
                for b in range(BPC):
                    front = _emit_front(nc, pools, aps, b, include_b1)
                    idxw = _emit_idxw(nc, pools, b, front)
                    for Q in range(NGRP):
                        _emit_group(nc, pools, aps, b, front, idxw, Q,
                                    include_b2)

    nc.compile()
    return nc


_CACHE = {}


def _get_module(include_b1, include_b2, reps=1):
    key = (include_b1, include_b2, reps)
    if key not in _CACHE:
        _CACHE[key] = _build(include_b1, include_b2, reps)
    return _CACHE[key]


def _split16(x):
    hi = x.astype(np.float16)
    lo = (x - hi.astype(np.float32)).astype(np.float16)
    return hi, lo


def _three_split16(x64):
    h1 = x64.astype(np.float16)
    r = x64 - h1.astype(np.float64)
    h2 = r.astype(np.float16)
    h3 = (r - h2.astype(np.float64)).astype(np.float16)
    return h1, h2, h3


def make_in_maps(xyz_dense, xyz_sparse, feat_sparse, W1, b1, W2, b2):
    xd = np.asarray(xyz_dense, np.float32)
    xs = np.asarray(xyz_sparse, np.float32)
    feat = np.asarray(feat_sparse, np.float32)

    # split-fp16 augmented score factors: S = 2 xd.xs - |xd|^2 - |xs|^2 = -d2
    # row pairing (lhs k <-> rhs k), 15 rows:
    #   k=0..2 coords c: (2xdh_c, xsh_c), (2xdh_c, xsl_c), (2xdl_c, xsh_c)
    #   9..11: (-nd_{h,m,l}, 1);  12..14: (1, -ns_{h,m,l})
    xdh, xdl = _split16(xd)
    xsh, xsl = _split16(xs)
    nd2 = np.sum(xd.astype(np.float64) ** 2, -1)
    ns2 = np.sum(xs.astype(np.float64) ** 2, -1)
    ndh, ndm, ndl = _three_split16(nd2)
    nsh, nsm, nsl = _three_split16(ns2)

    xdT = np.empty((B, KS, N2), np.float16)
    xsT = np.empty((B, KS, N1), np.float16)
    for c in range(3):
        xdT[:, 3 * c + 0] = 2.0 * xdh[:, :, c]
        xdT[:, 3 * c + 1] = 2.0 * xdh[:, :, c]
        xdT[:, 3 * c + 2] = 2.0 * xdl[:, :, c]
        xsT[:, 3 * c + 0] = xsh[:, :, c]
        xsT[:, 3 * c + 1] = xsl[:, :, c]
        xsT[:, 3 * c + 2] = xsh[:, :, c]
    xdT[:, 9], xdT[:, 10], xdT[:, 11] = -ndh, -ndm, -ndl
    xdT[:, 12:15] = 1.0
    xsT[:, 9:12] = 1.0
    xsT[:, 12], xsT[:, 13], xsT[:, 14] = -nsh, -nsm, -nsl

    featT = np.ascontiguousarray(feat.transpose(0, 2, 1)).astype(np.float16)
    # partition-major coords: [p, chunk, 3] with point index = chunk*128 + p
    xd_pc = np.ascontiguousarray(xd.reshape(B, NCH, 128, 3).transpose(0, 2, 1, 3))
    xs_pc = np.ascontiguousarray(xs.reshape(B, NR, 128, 3).transpose(0, 2, 1, 3))
    w1T = np.ascontiguousarray(np.asarray(W1, np.float32).T.astype(np.float16))
    w2T = np.ascontiguousarray(np.asarray(W2, np.float32).T.astype(np.float16))
    b1row = np.asarray(b1, np.float32).astype(np.float16).reshape(1, C_OUT)
    b2row = np.asarray(b2, np.float32).astype(np.float16).reshape(1, C_OUT)
    identm = np.eye(128, dtype=np.float16)

    in_maps = []
    for core in range(N_CORES):
        s = slice(core * BPC, (core + 1) * BPC)
        in_maps.append({
            "xdT": np.ascontiguousarray(xdT[s]),
            "xsT": np.ascontiguousarray(xsT[s]),
            "featT": np.ascontiguousarray(featT[s]),
            "xd_pc": np.ascontiguousarray(xd_pc[s]),
            "xs_pc": np.ascontiguousarray(xs_pc[s]),
            "w1T": w1T, "w2T": w2T, "b1row": b1row, "b2row": b2row,
            "ident": identm,
        })
    return in_maps


def kernel(xyz_dense, xyz_sparse, feat_sparse, W1, b1, W2, b2):
    include_b1 = bool(np.any(np.asarray(b1) != 0))
    include_b2 = bool(np.any(np.asarray(b2) != 0))
    nc = _get_module(include_b1, include_b2)
    in_maps = make_in_maps(xyz_dense, xyz_sparse, feat_sparse, W1, b1, W2, b2)
    res = run_bass_kernel_spmd(nc, in_maps, list(range(N_CORES)))
    out = np.concatenate([res.results[i]["out"] for i in range(N_CORES)], axis=0)
    return np.ascontiguousarray(out.astype(np.float32))
